# revision 27
# baseline (speedup 1.0000x reference)
"""Trainium2 Bass kernel for 16-head MHA (B=4, S=2048, E=1024), 8 NeuronCores.

Sharding: core c handles batch b = c//2 and head-group g = c%2 (8 heads each).
Tensor-parallel within the head group: column-parallel Wq/Wk/Wv, row-parallel
Wo; the two partial Wo outputs per batch are summed on the host.

All device matmuls run in fp16 with fp32 PSUM accumulation. Inputs are
pre-transposed on the host to feature-major layouts so every matmul contracts
over the partition dimension with no on-device transposes:
  QT/KT/VT  [E, S]   (feature, token)
  WqT/WkT/WvT [E, O] (in-feature, local out-feature), O = 512
  WoT       [O, E]   (local out-feature, out)
Output per core: OUT [E, S] fp16 = partial (Wo @ ctx^T) for this head group.

Schedule: the attention inner loop is ACT-bound (exp of S^2 scores), so the
PE-bound q/k projections for head-pair ob+1 are drip-fed into the PE queue
between attention matmuls of head-pair ob ("granule" filler), overlapping the
two bottleneck engines. Softmax normalization uses a DVE reciprocal plus a
GpSimd partition-broadcast (no PE/PSUM involvement) and runs per-ob under the
next ob's attention. Output row-blocks DMA out as soon as each is ready.
"""

import sys

sys.path.insert(0, "/opt/trn_rl_repo")

import numpy as np

# Problem constants (hardcoded; kernel.py must be self-contained).
B = 4
S = 2048
E = 1024
H = 16
D = 64
N_CORES = 8
HL = H // 2  # heads per core (head-group of 8)
O = HL * D  # 512 local output features of the q/k/v projections
IC = E // 128  # 8 contraction chunks for projections
OB = O // 128  # 4 output row-blocks (head pairs)
TB = S // 128  # 16 token blocks
KB = S // 128  # 16 key blocks per head
QCHUNK = 1024  # q columns processed per softmax tile
QC = S // QCHUNK  # 2
NV = 65  # v columns + 1 ones column for the softmax denominator

_CACHE = {}


def _build(phases="ABCD"):
    import concourse.bass as bass
    import concourse.mybir as mybir
    from concourse import bacc, tile

    f32 = mybir.dt.float32
    f16 = mybir.dt.float16
    Exp = mybir.ActivationFunctionType.Exp

    nc = bacc.Bacc(None, target_bir_lowering=False)

    # Packed inputs: per contraction chunk ic, the activation chunk [128, S]
    # followed by the matching projection-weight chunk [128, O].
    XW = S + O
    INQ = nc.dram_tensor("INQ", [128, IC * XW], f16, kind="ExternalInput")
    INK = nc.dram_tensor("INK", [128, IC * XW], f16, kind="ExternalInput")
    INV = nc.dram_tensor("INV", [128, IC * XW], f16, kind="ExternalInput")
    INW = nc.dram_tensor("INW", [128, OB * E], f16, kind="ExternalInput")
    OUT = nc.dram_tensor("OUT", [E, S], f16, kind="ExternalOutput")

    do_b = "B" in phases
    do_c = "C" in phases
    do_d = "D" in phases

    with tile.TileContext(nc) as tc:
        with (
            tc.tile_pool(name="weights", bufs=1) as wp,
            tc.tile_pool(name="qkv", bufs=1) as qkvp,
        ):
            wo_sb = wp.tile([128, OB, E], f16, tag="wo")

            # ---- persistent qT/kT/v in SBUF (f16) ----
            qt_sb = [qkvp.tile([128, S], f16, tag=f"qt{ob}", name=f"qt{ob}") for ob in range(OB)]
            kt_sb = [qkvp.tile([128, S], f16, tag=f"kt{ob}", name=f"kt{ob}") for ob in range(OB)]
            v_sb = [qkvp.tile([128, HL * NV], f16, tag=f"v{tb}", name=f"v{tb}") for tb in range(TB)]
            for tb in range(TB):
                ones_col = v_sb[tb].rearrange("p (h x) -> p h x", x=NV)[:, :, D : D + 1]
                nc.vector.memset(ones_col, 1.0)

            with tc.tile_pool(name="packqk", bufs=1) as packqk:
                inq = packqk.tile([128, IC, XW], f16, tag="inq")
                ink = packqk.tile([128, IC, XW], f16, tag="ink")
                # chunked input DMAs: per-ic pieces so the first projection
                # matmuls start as soon as chunk 0 lands on each ring
                for ic in range(IC):
                    nc.sync.dma_start(
                        inq[:, ic, :], INQ[:, ic * XW : (ic + 1) * XW]
                    )
                    nc.scalar.dma_start(
                        ink[:, ic, :], INK[:, ic * XW : (ic + 1) * XW]
                    )
                # WoT not needed until phase D — queue it BEHIND the packed
                # input stream on the sync ring
                nc.sync.dma_start(wo_sb[:].rearrange("p a b -> p (a b)"), INW[:])

                # ---- v input pack: weights first, then per-token-block
                # chunks, so v[tb] is computable as soon as its own slice
                # lands (token-major layout) ----
                packv = packqk  # same lifetime: inv slices feed B(ob0) filler
                inv = packv.tile([128, IC * O + TB * IC * 128], f16, tag="inv")
                invw = inv[:, 0 : IC * O].rearrange("p (a b) -> p a b", a=IC)
                invt = inv[:, IC * O :].rearrange(
                    "p (t a b) -> p t a b", t=TB, a=IC
                )
                nc.gpsimd.dma_start(inv[:, 0 : IC * O], INV[:, 0 : IC * O])
                for tb in range(TB):
                    o0 = IC * O + tb * IC * 128
                    nc.gpsimd.dma_start(
                        inv[:, o0 : o0 + IC * 128], INV[:, o0 : o0 + IC * 128]
                    )

                def v_steps(pool, t0, t1, tag="pp"):
                    for tb in range(t0, t1):
                        ps = pool.tile([128, O], f32, tag=tag, name=f"psv{tb}")
                        for ic in range(IC):
                            yield nc.tensor.matmul(
                                ps[:],
                                invt[:, tb, ic, :],
                                invw[:, ic, :],
                                start=(ic == 0),
                                stop=(ic == IC - 1),
                            )
                        vdst = v_sb[tb].rearrange("p (h x) -> p h x", x=NV)[:, :, 0:D]
                        yield nc.vector.tensor_copy(
                            vdst, ps[:].rearrange("p (h d) -> p h d", d=D)
                        )

                # ---- q/k projection granule generator ----
                def proj_steps(ob, pool):
                    for which, pk, dst in (("k", ink, kt_sb), ("q", inq, qt_sb)):
                        for j in range(S // 512):
                            ps = pool.tile(
                                [128, 512], f32, tag="pp", name=f"pp_{which}{ob}_{j}"
                            )
                            for ic in range(IC):
                                yield nc.tensor.matmul(
                                    ps[:],
                                    pk[:, ic, S + ob * 128 : S + (ob + 1) * 128],
                                    pk[:, ic, j * 512 : (j + 1) * 512],
                                    start=(ic == 0),
                                    stop=(ic == IC - 1),
                                )
                            yield nc.vector.tensor_copy(
                                dst[ob][:, j * 512 : (j + 1) * 512], ps[:]
                            )

                # prologue: k then q projections for ob0 ic-outer (paced
                # by the input DMA chunks) with v token blocks 0-7
                # interleaved to fill arrival gaps; v 8-15 are drip-fed
                # as filler during the first attention block
                with (
                    tc.tile_pool(name="psum_prok", bufs=4, space="PSUM") as prok,
                    tc.tile_pool(name="psum_prov", bufs=2, space="PSUM") as prov,
                ):
                    vgen_pro = v_steps(prov, 0, 8, tag="pv")
                    for which, pk, dst in (("k", ink, kt_sb), ("q", inq, qt_sb)):
                        tiles = [
                            prok.tile(
                                [128, 512], f32, tag="kq", name=f"pro_{which}{j}"
                            )
                            for j in range(4)
                        ]
                        for ic in range(IC):
                            for j in range(4):
                                nc.tensor.matmul(
                                    tiles[j][:],
                                    pk[:, ic, S : S + 128],
                                    pk[:, ic, j * 512 : (j + 1) * 512],
                                    start=(ic == 0),
                                    stop=(ic == IC - 1),
                                )
                            if which == "k":
                                for _ in range(9):
                                    next(vgen_pro, None)
                        for j in range(4):
                            nc.vector.tensor_copy(
                                dst[0][:, j * 512 : (j + 1) * 512], tiles[j][:]
                            )
                    for _ in vgen_pro:
                        pass

                # ======== attention (+ next-ob projections as filler) ========
                with (
                    tc.tile_pool(name="cun", bufs=1) as cunp,
                    tc.tile_pool(name="bcp", bufs=1) as bcp,
                    tc.tile_pool(name="rcp", bufs=1) as rcp,
                    tc.tile_pool(name="attn", bufs=3) as attnp,
                    tc.tile_pool(name="psum_s", bufs=2, space="PSUM") as pss_pool,
                    tc.tile_pool(name="psum_c", bufs=1, space="PSUM") as psc_pool,
                    tc.tile_pool(name="psum_p", bufs=2, space="PSUM") as psp_pool,
                ):
                    ctxt_sb = [None] * OB

                    class Filler:
                        def __init__(self, *gens):
                            self.gens = list(gens)

                        def pull(self, n=1):
                            for _ in range(n):
                                while self.gens:
                                    if next(self.gens[0], None) is None:
                                        self.gens.pop(0)
                                    else:
                                        break

                        def drain(self):
                            for g in self.gens:
                                for _ in g:
                                    pass
                            self.gens = []

                    for ob in range(OB if do_b else 0):
                        gens = [v_steps(psp_pool, 8, TB)] if ob == 0 else []
                        if ob + 1 < OB:
                            gens.append(proj_steps(ob + 1, psp_pool))
                        filler = Filler(*gens)
                        # unnormalized ctx^T for this ob (f16), freed after
                        # the normalize multiply — single rotating slot
                        cu = cunp.tile([128, S], f16, tag="cu", name=f"cu{ob}")
                        # one full-height broadcast tile per head: HW
                        # partition_broadcast ignores output partition offsets,
                        # so each head's reciprocal fills all 128 rows and the
                        # multiply picks the right 64-row half.
                        bc2 = (
                            [
                                bcp.tile([128, S], f16, tag=f"bc{t}", name=f"bc{ob}_{t}")
                                for t in range(2)
                            ]
                            if do_c
                            else None
                        )
                        for hl in (2 * ob, 2 * ob + 1):
                            r0 = (hl % 2) * 64
                            for qc in range(QC):
                                q0 = qc * QCHUNK
                                pc = psc_pool.tile(
                                    [NV, QCHUNK], f32, tag="pc", name=f"pc{hl}_{qc}"
                                )
                                ats = [None] * KB
                                for kb in range(KB):
                                    ps = pss_pool.tile(
                                        [128, QCHUNK], f32, tag="ps", name=f"sc{hl}_{qc}_{kb}"
                                    )
                                    at = attnp.tile(
                                        [128, QCHUNK], f16, tag="at", name=f"at{hl}_{qc}_{kb}"
                                    )
                                    for j in range(QCHUNK // 512):
                                        nc.tensor.matmul(
                                            ps[:, j * 512 : (j + 1) * 512],
                                            kt_sb[ob][r0 : r0 + 64, kb * 128 : (kb + 1) * 128],
                                            qt_sb[ob][r0 : r0 + 64, q0 + j * 512 : q0 + (j + 1) * 512],
                                            start=True,
                                            stop=True,
                                        )
                                    nc.scalar.activation(at[:], ps[:], Exp, scale=0.125)
                                    ats[kb] = at
                                    # attn@V skewed two steps behind the
                                    # scores: the PE never waits on the
                                    # current exp, and the first av of a block
                                    # lands after the previous block's pc
                                    # drain has released the psum slot
                                    if kb >= 2:
                                        for j in range(QCHUNK // 512):
                                            nc.tensor.matmul(
                                                pc[:, j * 512 : (j + 1) * 512],
                                                v_sb[kb - 2][:, hl * NV : (hl + 1) * NV],
                                                ats[kb - 2][:, j * 512 : (j + 1) * 512],
                                                start=(kb - 2 == 0),
                                                stop=False,
                                            )
                                        ats[kb - 2] = None
                                    filler.pull(
                                        9 if (hl == 0 and qc == 0 and kb < 8) else 1
                                    )
                                for kbt in (KB - 2, KB - 1):
                                    for j in range(QCHUNK // 512):
                                        nc.tensor.matmul(
                                            pc[:, j * 512 : (j + 1) * 512],
                                            v_sb[kbt][:, hl * NV : (hl + 1) * NV],
                                            ats[kbt][:, j * 512 : (j + 1) * 512],
                                            start=False,
                                            stop=(kbt == KB - 1),
                                        )
                                    filler.pull(1)
                                nc.vector.tensor_copy(
                                    cu[r0 : r0 + 64, q0 : q0 + QCHUNK], pc[0:D, :]
                                )
                                if do_c:
                                    # reciprocal of the denominator row straight
                                    # from PSUM, broadcast into the bc tile for
                                    # this head (DVE + Pool only, no PE)
                                    rq = rcp.tile(
                                        [1, QCHUNK], f16, tag="rq", name=f"rq{hl}_{qc}"
                                    )
                                    with nc.allow_low_precision(
                                        reason="softmax recip in f16"
                                    ):
                                        nc.vector.reciprocal(rq[:], pc[D : D + 1, :])
                                    nc.gpsimd.partition_broadcast(
                                        bc2[hl % 2][:, q0 : q0 + QCHUNK],
                                        rq[:],
                                        channels=128,
                                    )
                        # drain any remaining granules before the next block
                        filler.drain()
                        # ---- normalize ctx^T for this ob (no PE/PSUM) ----
                        if do_c:
                            ctxt_sb[ob] = qkvp.tile(
                                [128, S], f16, tag=f"kt{ob}", name=f"ctxt{ob}"
                            )
                            for t in range(2):
                                r0 = 64 * t
                                nc.vector.tensor_mul(
                                    ctxt_sb[ob][r0 : r0 + 64, :],
                                    cu[r0 : r0 + 64, :],
                                    bc2[t][r0 : r0 + 64, :],
                                )

            # ================= output projection =================
            with (
                tc.tile_pool(name="outs", bufs=3) as outsp,
                tc.tile_pool(name="psum_o", bufs=2, space="PSUM") as pso,
            ):
                rings = (nc.sync, nc.scalar, nc.gpsimd)
                outv = OUT[:].rearrange("(eb p) s -> p eb s", p=128)
                for eb in range(E // 128 if do_d else 0):
                    po = pso.tile([128, S], f32, tag="po", name=f"po{eb}")
                    for oc in range(OB):
                        for j in range(S // 512):
                            nc.tensor.matmul(
                                po[:, j * 512 : (j + 1) * 512],
                                wo_sb[:, oc, eb * 128 : (eb + 1) * 128],
                                ctxt_sb[oc][:, j * 512 : (j + 1) * 512],
                                start=(oc == 0),
                                stop=(oc == OB - 1),
                            )
                    so = outsp.tile([128, S], f16, tag="so", name=f"so{eb}")
                    nc.scalar.copy(so[:], po[:])
                    rings[eb % 3].dma_start(outv[:, eb, :], so[:])

    nc.compile()
    return nc


def _get_nc():
    if "nc" not in _CACHE:
        _CACHE["nc"] = _build()
    return _CACHE["nc"]


def _shard_inputs(Q, K, V, Wq, Wk, Wv, Wo):
    f16 = np.float16
    Q = np.asarray(Q, np.float32)
    K = np.asarray(K, np.float32)
    V = np.asarray(V, np.float32)
    Wq = np.asarray(Wq, np.float32)
    Wk = np.asarray(Wk, np.float32)
    Wv = np.asarray(Wv, np.float32)
    Wo = np.asarray(Wo, np.float32)

    def pack_xw(XTb, WTg):
        # [128, IC*(S+O)]: per chunk ic, activation chunk then weight chunk
        parts = []
        for ic in range(IC):
            parts.append(XTb[ic * 128 : (ic + 1) * 128, :])
            parts.append(WTg[ic * 128 : (ic + 1) * 128, :])
        return np.ascontiguousarray(np.concatenate(parts, axis=1)).astype(f16)

    def pack_v(XTb, WTg):
        # [128, IC*O + TB*IC*128]: all weight chunks first, then per token
        # block tb the IC activation sub-chunks for those 128 tokens
        parts = [WTg[ic * 128 : (ic + 1) * 128, :] for ic in range(IC)]
        for tb in range(TB):
            for ic in range(IC):
                parts.append(
                    XTb[ic * 128 : (ic + 1) * 128, tb * 128 : (tb + 1) * 128]
                )
        return np.ascontiguousarray(np.concatenate(parts, axis=1)).astype(f16)

    in_maps = []
    for c in range(N_CORES):
        b, g = divmod(c, 2)
        sl = slice(g * O, (g + 1) * O)
        wot = Wo[:, sl].T  # [O, E]
        inw = np.concatenate(
            [wot[oc * 128 : (oc + 1) * 128, :] for oc in range(OB)], axis=1
        )
        in_maps.append(
            {
                "INQ": pack_xw(Q[b].T, Wq[sl, :].T),
                "INK": pack_xw(K[b].T, Wk[sl, :].T),
                "INV": pack_v(V[b].T, Wv[sl, :].T),
                "INW": np.ascontiguousarray(inw).astype(f16),
            }
        )
    return in_maps


class _Runner:
    """Compile-once executor for the SPMD bass program on 8 cores.

    Mirrors concourse.bass2jax.run_bass_via_pjrt but hoists the jit out of
    the call so repeated invocations don't re-trace/re-lower. With
    donate=False the output-shaped operands are not consumed, so calls can be
    chained (feeding outputs back in) to measure marginal device time.
    """

    def __init__(self, nc, donate=True):
        import jax
        import concourse.mybir as mybir
        from concourse import bass2jax

        bass2jax.install_neuronx_cc_hook()
        self.jax = jax
        self.nc = nc
        partition_name = (
            nc.partition_id_tensor.name if nc.partition_id_tensor else None
        )
        in_names, out_names, out_avals = [], [], []
        for alloc in nc.m.functions[0].allocations:
            if not isinstance(alloc, mybir.MemoryLocationSet):
                continue
            name = alloc.memorylocations[0].name
            if alloc.kind == "ExternalInput":
                if name != partition_name:
                    in_names.append(name)
            elif alloc.kind == "ExternalOutput":
                out_names.append(name)
                out_avals.append(
                    jax.core.ShapedArray(
                        tuple(alloc.tensor_shape), mybir.dt.np(alloc.dtype)
                    )
                )
        self.in_names = in_names
        self.out_names = out_names
        self.out_avals = out_avals
        n_params = len(in_names)
        n_outs = len(out_names)
        all_in_names = list(in_names) + list(out_names)
        if partition_name is not None:
            all_in_names.append(partition_name)
        all_in_names = tuple(all_in_names)

        def _body(*args):
            operands = list(args)
            if partition_name is not None:
                operands.append(bass2jax.partition_id_tensor())
            outs = bass2jax._bass_exec_p.bind(
                *operands,
                out_avals=tuple(out_avals),
                in_names=all_in_names,
                out_names=tuple(out_names),
                lowering_input_output_aliases=(),
                sim_require_finite=True,
                sim_require_nnan=True,
                nc=nc,
            )
            return tuple(outs)

        from concourse.bass2jax import Mesh, PartitionSpec, shard_map

        devices = jax.devices()[:N_CORES]
        mesh = Mesh(np.asarray(devices), ("core",))
        self.mesh = mesh
        self.pspec = PartitionSpec("core")
        self.sharded = jax.jit(
            shard_map(
                _body,
                mesh=mesh,
                in_specs=(PartitionSpec("core"),) * (n_params + n_outs),
                out_specs=(PartitionSpec("core"),) * n_outs,
                check_rep=False,
            ),
            donate_argnums=(
                tuple(range(n_params, n_params + n_outs)) if donate else ()
            ),
            keep_unused=True,
        )

    def concat_inputs(self, in_maps):
        return [
            np.concatenate([np.asarray(m[name]) for m in in_maps], axis=0)
            for name in self.in_names
        ]

    def zero_outs(self):
        return [
            np.zeros((N_CORES * a.shape[0], *a.shape[1:]), a.dtype)
            for a in self.out_avals
        ]

    def __call__(self, concat_in, concat_zeros=None):
        if concat_zeros is None:
            concat_zeros = self.zero_outs()
        out_arrs = self.sharded(*concat_in, *concat_zeros)
        return [
            {
                name: np.asarray(out_arrs[i]).reshape(
                    N_CORES, *self.out_avals[i].shape
                )[c]
                for i, name in enumerate(self.out_names)
            }
            for c in range(N_CORES)
        ]


def _get_runner():
    if "runner" not in _CACHE:
        _CACHE["runner"] = _Runner(_get_nc())
    return _CACHE["runner"]


def kernel(Q, K, V, mask, Wq, Wk, Wv, Wo):
    runner = _get_runner()
    in_maps = _shard_inputs(Q, K, V, Wq, Wk, Wv, Wo)
    results = runner(runner.concat_inputs(in_maps))
    out = np.empty((B, S, E), np.float32)
    for b in range(B):
        acc = results[2 * b]["OUT"].astype(np.float32) + results[2 * b + 1][
            "OUT"
        ].astype(np.float32)
        out[b] = acc.T
    return out
